# revision 56
# baseline (speedup 1.0000x reference)
# Trainium2 Bass kernel for DeepSeek-style sparse attention.
# Self-contained: hardcodes shapes from the problem spec.
#   x [1, 2048, 768]; Wq/Wk/Wv/Wo [768, 768]; biases [768]; Ws [12, 768]; bs [12]
#
# Sharding: 8 cores = 4 query-blocks (512 queries) x 2 head-groups (6 heads).
# Each core computes, for its 512 queries and 6 heads:
#   - band K/Q/V projections over an 8-chunk (1024-token) window that covers
#     the +-256 local band of its query block (host slices x accordingly),
#   - a compact top-k branch from host-gathered x columns (256 per head,
#     indices from a tiny fp32 phase-A token-score kernel + host argpartition),
#   - the 16-token global branch,
#   - a partial out-projection over its 6 heads' dims. The host sums the two
#     head-group partials per query block and adds bo (no device collective).
# Everything post-PSUM runs in bf16; phase A stays fp32 so the top-k
# selection matches the fp32 reference bit-for-bit on near-ties.
import sys
import numpy as np
import ml_dtypes

sys.path.insert(0, "/opt/trn_rl_repo")

import concourse.bass as bass
from concourse import bacc
import concourse.mybir as mybir
from concourse.tile import TileContext
from concourse.bass_utils import run_bass_kernel_spmd
from concourse.masks import make_identity

S = 2048
D = 768
H = 12
DH = 64
NCORES = 8
NQB = 4                    # query blocks
QB = S // NQB              # 512 queries per block
HG = 2                     # head groups
HPG = H // HG              # 6 heads per group
ECH = D // 128             # 6 embedding chunks
BCH = 8                    # band chunks per core (8 x 128 = 1024 tokens)
BT = BCH * 128             # 1024 band tokens
TOPK = 256
NG = 16
LWH = 256                  # local window half-width
SCALE = 1.0 / np.sqrt(DH)
F32 = mybir.dt.float32
BF16 = mybir.dt.bfloat16
FP8 = mybir.dt.float8e4
Exp = mybir.ActivationFunctionType.Exp
Ident = mybir.ActivationFunctionType.Identity
DblRow = mybir.MatmulPerfMode.DoubleRow


def _patch_tile_drain():
    """This walrus build rejects sem-waits on Drain instructions ("Too many
    sync wait commands"). Emit the tail waits as individual SemWait ops on
    the sync engine instead, then a bare drain."""
    if getattr(TileContext, "_drain_patched", False):
        return

    def _drain_and_barrier(self, tick_clock, wait_clock):
        nc = self.nc
        clock = tick_clock.global_clock
        for proc, handle in sorted(self.sems.allocated().items()):
            tick = clock[proc]
            if tick <= 0:
                continue
            mult = 16 if "DMA" in handle.name else 1
            nc.sync.wait_ge(handle, tick * mult)
        nc.sync.drain()
        nc.all_engine_barrier()
        popped = nc._tile_sem_poison_stack.pop()
        assert popped is self._sem_poison
        nc.clear_and_free_semaphores(list(self.sems.allocated().values()))
        nc.all_engine_barrier()

    TileContext._drain_and_barrier = _drain_and_barrier
    TileContext._drain_patched = True


def _build_phase_a():
    """ts[h, 256] = (Ws @ x^T + bs) for this core's 256-token slice.

    Runs in bf16; the host re-evaluates tokens near the top-k threshold in
    exact fp32, so rounding cannot flip the selection."""
    _patch_tile_drain()
    nc = bacc.Bacc()
    TPC = S // NCORES  # 256 tokens per core
    xTa = nc.declare_dram_parameter("xTa", [128, ECH, TPC], BF16, isOutput=False)
    WsT = nc.declare_dram_parameter("WsT", [128, ECH, H], BF16, isOutput=False)
    bs_row = nc.declare_dram_parameter("bs_row", [1, H], BF16, isOutput=False)
    ts = nc.declare_dram_parameter("ts", [H, TPC], F32, isOutput=True)

    with TileContext(nc) as tc, nc.allow_low_precision(reason="host refines boundary"):
        with (
            tc.tile_pool(name="sb", bufs=1) as sb,
            tc.tile_pool(name="ps", bufs=1, space="PSUM") as ps,
        ):
            xTa_sb = sb.tile([128, ECH, TPC], BF16)
            WsT_sb = sb.tile([128, ECH, H], BF16)
            bs_sb = sb.tile([1, H], BF16)
            ones = sb.tile([1, TPC], BF16)
            nc.vector.memset(ones, 1.0)
            nc.sync.dma_start(out=bs_sb, in_=bs_row[:, :])
            nc.sync.dma_start(out=WsT_sb, in_=WsT[:, :, :])
            nc.sync.dma_start(out=xTa_sb, in_=xTa[:, :, :])
            acc = ps.tile([H, TPC], F32)
            for ec in range(ECH):
                nc.tensor.matmul(
                    acc, WsT_sb[:, ec, :], xTa_sb[:, ec, :],
                    start=(ec == 0), stop=False,
                )
            nc.tensor.matmul(acc, bs_sb, ones, start=False, stop=True)
            ts_sb = sb.tile([H, TPC], F32)
            nc.vector.tensor_copy(ts_sb, acc)
            nc.sync.dma_start(out=ts[:, :], in_=ts_sb)
    nc.finalize()
    return nc


def _build_phase_b(debug=False):
    """Per-core sparse attention for 512 queries x 6 heads (see header)."""
    _patch_tile_drain()
    nc = bacc.Bacc()
    # host-prepared, partition-major, bf16
    xTb = nc.declare_dram_parameter("xTb", [128, ECH, BT], BF16, isOutput=False)
    xTq = nc.declare_dram_parameter("xTq", [128, ECH, QB], BF16, isOutput=False)
    xTk = nc.declare_dram_parameter("xTk", [128, ECH, HPG * TOPK], BF16, isOutput=False)
    xTg = nc.declare_dram_parameter("xTg", [128, ECH, NG], BF16, isOutput=False)
    # per-type weight slices for this head group (384 = 6 heads x 64 dims)
    WqTg = nc.declare_dram_parameter("WqTg", [128, ECH, 384], BF16, isOutput=False)
    WkTg = nc.declare_dram_parameter("WkTg", [128, ECH, 384], BF16, isOutput=False)
    WvTg = nc.declare_dram_parameter("WvTg", [128, ECH, 384], BF16, isOutput=False)
    # per-partition bias columns, added during the PSUM->SBUF copies
    bqg_col = nc.declare_dram_parameter("bqg_col", [128, HPG // 2], F32, isOutput=False)
    bkg_col = nc.declare_dram_parameter("bkg_col", [128, HPG // 2], F32, isOutput=False)
    bkv2_col = nc.declare_dram_parameter("bkv2_col", [128, HPG], F32, isOutput=False)
    bvg = nc.declare_dram_parameter("bvg", [1, 384], BF16, isOutput=False)
    # stacked [Wk_h | Wv_h] (even local head) / [Wv_h | Wk_h] (odd) for the
    # compact top-k / global projections
    Wkv2 = nc.declare_dram_parameter("Wkv2", [128, ECH, HPG, 128], BF16, isOutput=False)
    # Wo^T/3 slices: wo[dh, j, dc, :] over this group's heads
    Wo3 = nc.declare_dram_parameter("Wo3", [DH, HPG, ECH, 128], BF16, isOutput=False)
    M8 = nc.declare_dram_parameter("M8", [128, BCH, QB], BF16, isOutput=False)
    yT = nc.declare_dram_parameter("yT", [128, ECH, QB], F32, isOutput=True)

    with TileContext(nc) as tc, nc.allow_low_precision(reason="bf16 validated vs reference"):
        with tc.tile_pool(name="perm", bufs=1) as perm:
            kT_sb = perm.tile([128, HPG // 2, BT], BF16)       # 2 heads / 128 parts
            qT_sb = perm.tile([128, HPG // 2, QB], BF16)
            ktk_sb = perm.tile([128, HPG // 2, TOPK], BF16)
            vtkT_sb = perm.tile([128, HPG // 2, TOPK], BF16)   # staging (pre-transpose)
            vtk_sb = perm.tile([128, HPG, TOPK // 128, DH], FP8)
            kg_sb = perm.tile([128, HPG // 2, NG], BF16)
            vgT_sb = perm.tile([128, HPG // 2, NG], BF16)      # staging
            vg_sb = perm.tile([NG, HPG, DH], BF16)
            v_sb = perm.tile([128, HPG, BCH, DH], FP8)
            M8_sb = perm.tile([128, BCH, QB], BF16)
            attnT = perm.tile([DH, HPG, QB], BF16)
            yT_sb = perm.tile([128, ECH, QB], F32)
            wo_sb = perm.tile([DH, HPG, ECH, 128], BF16)
            bqg_sb = perm.tile([128, HPG // 2], F32)
            bkg_sb = perm.tile([128, HPG // 2], F32)
            bvg_sb = perm.tile([1, 384], BF16)
            bkv2_sb = perm.tile([128, HPG], F32)
            ones = perm.tile([1, QB], BF16)
            id128 = perm.tile([128, 128], BF16)
            nc.vector.memset(ones, 1.0)
            make_identity(nc, id128)
            ones8 = perm.tile([128, 2, DH], FP8)
            nc.vector.memset(ones8, 1.0)
            ones16 = perm.tile([NG, DH], BF16)
            nc.vector.memset(ones16, 1.0)
            nc.sync.dma_start(out=bqg_sb, in_=bqg_col[:, :])
            nc.sync.dma_start(out=bkg_sb, in_=bkg_col[:, :])
            nc.sync.dma_start(out=bvg_sb, in_=bvg[:, :])
            nc.sync.dma_start(out=bkv2_sb, in_=bkv2_col[:, :])

            with (
                tc.tile_pool(name="xin", bufs=1) as xin,
                tc.tile_pool(name="pj_ps", bufs=4, space="PSUM") as pj_ps,
                tc.tile_pool(name="pg_ps", bufs=2, space="PSUM") as pg_ps,
                tc.tile_pool(name="pt_ps", bufs=2, space="PSUM") as pt_ps,
            ):
                xTb_sb = xin.tile([128, ECH, BT], BF16)
                xTq_sb = xin.tile([128, ECH, QB], BF16)
                xTk_sb = xin.tile([128, ECH, HPG * TOPK], BF16)
                xTg_sb = xin.tile([128, ECH, NG], BF16)
                WqTg_sb = xin.tile([128, ECH, 384], BF16)
                WkTg_sb = xin.tile([128, ECH, 384], BF16)
                WvTg_sb = xin.tile([128, ECH, 384], BF16)
                Wkv2_sb = xin.tile([128, ECH, HPG, 128], BF16)
                # Whole-tile DMAs (one descriptor per partition — per-chunk
                # splits multiply descriptor count and serialize the queues).
                # Q first so the PE starts while K/V/band stream in.
                nc.sync.dma_start(out=WqTg_sb, in_=WqTg[:, :, :])
                nc.sync.dma_start(out=xTq_sb, in_=xTq[:, :, :])
                nc.sync.dma_start(out=WkTg_sb, in_=WkTg[:, :, :])
                nc.sync.dma_start(out=xTb_sb, in_=xTb[:, :, :])
                nc.sync.dma_start(out=WvTg_sb, in_=WvTg[:, :, :])
                nc.sync.dma_start(out=Wkv2_sb, in_=Wkv2[:, :, :, :])
                nc.sync.dma_start(out=xTk_sb, in_=xTk[:, :, :])
                nc.sync.dma_start(out=xTg_sb, in_=xTg[:, :, :])
                # late-phase inputs last so they don't delay the projections
                nc.sync.dma_start(out=M8_sb, in_=M8[:, :, :])
                nc.sync.dma_start(out=wo_sb, in_=Wo3[:, :, :, :])

                # ---- Q^T [dims, 512], then band K^T [dims, 1024] ----
                # (biases added during the PSUM->SBUF copy on the ACT engine)
                for dc in range(HPG // 2):
                    ds = slice(128 * dc, 128 * (dc + 1))
                    qp = pj_ps.tile([128, 512], F32, tag="pj")
                    for ec in range(ECH):
                        nc.tensor.matmul(
                            qp, WqTg_sb[:, ec, ds], xTq_sb[:, ec, :],
                            start=(ec == 0), stop=(ec == ECH - 1),
                        )
                    nc.scalar.activation(
                        qT_sb[:, dc, :], qp, Ident, bias=bqg_sb[:, dc : dc + 1]
                    )
                for dc in range(HPG // 2):
                    ds = slice(128 * dc, 128 * (dc + 1))
                    for half in range(2):
                        t0 = 512 * half
                        kp = pj_ps.tile([128, 512], F32, tag="pj")
                        for ec in range(ECH):
                            nc.tensor.matmul(
                                kp, WkTg_sb[:, ec, ds],
                                xTb_sb[:, ec, t0 : t0 + 512],
                                start=(ec == 0), stop=(ec == ECH - 1),
                            )
                        nc.scalar.activation(
                            kT_sb[:, dc, t0 : t0 + 512], kp, Ident,
                            bias=bkg_sb[:, dc : dc + 1],
                        )

                # ---- band V [1024 tokens, 6 heads x 64] ----
                for tcn in range(BCH):
                    t0 = 128 * tcn
                    vp = pj_ps.tile([128, HPG * DH], F32, tag="pj")
                    for ec in range(ECH):
                        nc.tensor.matmul(
                            vp, xTb_sb[:, ec, t0 : t0 + 128],
                            WvTg_sb[:, ec, :],
                            start=(ec == 0), stop=False,
                        )
                    nc.tensor.matmul(
                        vp, ones[:, :128], bvg_sb,
                        start=False, stop=True,
                    )
                    nc.any.tensor_copy(
                        v_sb[:, :, tcn, :],
                        vp.rearrange("p (h d) -> p h d", d=DH),
                    )

                # ---- compact top-k + global K/V per head (stacked K|V) ----
                for j in range(HPG):
                    hp = (j % 2) * 64
                    dc = j // 2
                    sp = pj_ps.tile([128, TOPK], F32, tag="pj")
                    sg = pg_ps.tile([128, NG], F32, tag="pg")
                    for ec in range(ECH):
                        nc.tensor.matmul(
                            sp, Wkv2_sb[:, ec, j, :],
                            xTk_sb[:, ec, TOPK * j : TOPK * (j + 1)],
                            start=(ec == 0), stop=(ec == ECH - 1),
                        )
                        nc.tensor.matmul(
                            sg, Wkv2_sb[:, ec, j, :], xTg_sb[:, ec, :],
                            start=(ec == 0), stop=(ec == ECH - 1),
                        )
                    # even j: K at rows 0:64, V^T at 64:128; odd j: swapped
                    bj = bkv2_sb[:, j : j + 1]
                    nc.scalar.activation(
                        ktk_sb[hp : hp + 64, dc, :], sp[hp : hp + 64, :],
                        Ident, bias=bj[hp : hp + 64, :],
                    )
                    nc.scalar.activation(
                        kg_sb[hp : hp + 64, dc, :], sg[hp : hp + 64, :],
                        Ident, bias=bj[hp : hp + 64, :],
                    )
                    vq = 64 - hp
                    nc.scalar.activation(
                        vtkT_sb[vq : vq + 64, dc, :], sp[vq : vq + 64, :],
                        Ident, bias=bj[vq : vq + 64, :],
                    )
                    nc.scalar.activation(
                        vgT_sb[vq : vq + 64, dc, :], sg[vq : vq + 64, :],
                        Ident, bias=bj[vq : vq + 64, :],
                    )

                # ---- transpose V^T staging into [token, dh] layout ----
                for j in range(HPG):
                    vq = 64 - (j % 2) * 64
                    dc = j // 2
                    idsl = id128[vq : vq + 64, vq : vq + 64]
                    for c in range(TOPK // 128):
                        tp = pt_ps.tile([128, 64], BF16, tag="pt")
                        nc.tensor.transpose(
                            tp, vtkT_sb[vq : vq + 64, dc, 128 * c : 128 * (c + 1)],
                            idsl,
                        )
                        nc.any.tensor_copy(vtk_sb[:, j, c, :], tp)
                    tg = pt_ps.tile([128, 64], BF16, tag="pt")
                    nc.tensor.transpose(tg[0:NG, :], vgT_sb[vq : vq + 64, dc, :NG], idsl)
                    nc.any.tensor_copy(vg_sb[:, j, :], tg[0:NG, :])

            # ---- per-head attention ----
            # Scores stream through single-bank PSUM rounds -> exp (ACT) ->
            # band-mask multiply into fp8 (DVE). AV runs as fp8 DoubleRow
            # pairs: value passes into avv[0:64]; denominator passes against
            # an all-ones stationary into avd (already replicated across 64
            # partitions, so no broadcast step is needed). reciprocal reads
            # avd straight from PSUM at partition 0; the weighted sum reads
            # avv from PSUM. The whole normalization runs off the PE.
            with (
                tc.tile_pool(name="attn", bufs=2) as attn,
                tc.tile_pool(name="nrm", bufs=2) as nrm,
                tc.tile_pool(name="st_ps", bufs=2, space="PSUM") as st_ps,
                tc.tile_pool(name="av_ps", bufs=1, space="PSUM") as av_ps,
            ):
                for j in range(HPG):
                    hp = (j % 2) * 64
                    dc = j // 2
                    kTh = kT_sb[hp : hp + 64, dc, :]
                    qTh = qT_sb[hp : hp + 64, dc, :]
                    stg = st_ps.tile([128, QB], F32, tag="st")
                    nc.tensor.matmul(
                        stg[0:NG, :], kg_sb[hp : hp + 64, dc, :], qTh,
                        start=True, stop=True,
                    )
                    ETg = attn.tile([NG, QB], BF16, tag="ETg")
                    nc.scalar.activation(ETg, stg[0:NG, :], Exp, scale=SCALE)
                    ETk = attn.tile([128, 2, QB], FP8, tag="ETk")
                    for c in range(2):
                        stk = st_ps.tile([128, QB], F32, tag="st")
                        nc.tensor.matmul(
                            stk, ktk_sb[hp : hp + 64, dc, 128 * c : 128 * (c + 1)],
                            qTh, start=True, stop=True,
                        )
                        nc.scalar.activation(ETk[:, c, :], stk, Exp, scale=SCALE)
                    ET = attn.tile([128, BCH, QB], BF16, tag="ET")
                    EB8 = attn.tile([128, BCH, QB], FP8, tag="EB8")
                    for cc in range(BCH):
                        stp = st_ps.tile([128, QB], F32, tag="st")
                        nc.tensor.matmul(
                            stp, kTh[:, 128 * cc : 128 * (cc + 1)], qTh,
                            start=True, stop=True,
                        )
                        nc.scalar.activation(ET[:, cc, :], stp, Exp, scale=SCALE)
                        nc.vector.tensor_mul(
                            EB8[:, cc, :], ET[:, cc, :], M8_sb[:, cc, :]
                        )
                    avv = av_ps.tile([DH, 3, QB], F32, tag="avv")
                    avd = av_ps.tile([DH, 3, QB], F32, tag="avd")
                    rbs = nrm.tile([DH, 3, QB], F32, tag="rbs")
                    nc.tensor.matmul(
                        avv[:, 2, :], vg_sb[:, j, :], ETg, start=True, stop=True
                    )
                    nc.tensor.matmul(
                        avd[:, 2, :], ones16, ETg, start=True, stop=True
                    )
                    nc.vector.reciprocal_approx_fast(rbs[:, 2, :], avd[:, 2, :])
                    nc.tensor.matmul(
                        avv[:, 1, :], vtk_sb[:, j, :, :], ETk,
                        perf_mode=DblRow, start=True, stop=True,
                    )
                    nc.tensor.matmul(
                        avd[:, 1, :], ones8, ETk,
                        perf_mode=DblRow, start=True, stop=True,
                    )
                    nc.vector.reciprocal_approx_fast(rbs[:, 1, :], avd[:, 1, :])
                    for t2 in range(BCH // 2):
                        nc.tensor.matmul(
                            avv[:, 0, :], v_sb[:, j, 2 * t2 : 2 * t2 + 2, :],
                            EB8[:, 2 * t2 : 2 * t2 + 2, :],
                            perf_mode=DblRow,
                            start=(t2 == 0), stop=(t2 == BCH // 2 - 1),
                        )
                    for t2 in range(BCH // 2):
                        nc.tensor.matmul(
                            avd[:, 0, :], ones8,
                            EB8[:, 2 * t2 : 2 * t2 + 2, :],
                            perf_mode=DblRow,
                            start=(t2 == 0), stop=(t2 == BCH // 2 - 1),
                        )
                    nc.vector.reciprocal_approx_fast(rbs[:, 0, :], avd[:, 0, :])
                    ta = nrm.tile([DH, QB], F32, tag="ta")
                    tb = nrm.tile([DH, QB], F32, tag="tb")
                    nc.vector.tensor_mul(ta, avv[:, 2, :], rbs[:, 2, :])
                    nc.vector.tensor_mul(tb, avv[:, 1, :], rbs[:, 1, :])
                    nc.vector.tensor_add(ta, ta, tb)
                    nc.vector.tensor_mul(tb, avv[:, 0, :], rbs[:, 0, :])
                    nc.vector.tensor_add(attnT[:, j, :], ta, tb)

            # ---- partial out-projection (Wo/3 folded on host) ----
            with tc.tile_pool(name="yt_ps", bufs=2, space="PSUM") as yt_ps:
                for ddc in range(ECH):
                    yp = yt_ps.tile([128, QB], F32, tag="yt")
                    for j in range(HPG):
                        nc.tensor.matmul(
                            yp, wo_sb[:, j, ddc, :], attnT[:, j, :],
                            start=(j == 0), stop=(j == HPG - 1),
                        )
                    nc.any.tensor_copy(yT_sb[:, ddc, :], yp)
                    nc.sync.dma_start(out=yT[:, ddc, :], in_=yT_sb[:, ddc, :])
    nc.finalize()
    return nc


_PROGS = {}
TRACE = False
LAST_EXEC_NS = {}


def _get_progs():
    if "a" not in _PROGS:
        _PROGS["a"] = _build_phase_a()
        _PROGS["b"] = _build_phase_b()
    return _PROGS["a"], _PROGS["b"]


def _pm(arr, dtype):
    """[768, T] -> partition-major [128, 6, T] contiguous."""
    d, t = arr.shape
    assert d == D
    return np.ascontiguousarray(
        arr.reshape(ECH, 128, t).transpose(1, 0, 2).astype(dtype)
    )


def _inputs_b(inputs, xT, topk_idx):
    """Build the 8 per-core phase-B input maps."""
    bf = ml_dtypes.bfloat16
    WqT = inputs["Wq"].T.astype(np.float32)
    WkT = inputs["Wk"].T.astype(np.float32)
    WvT = inputs["Wv"].T.astype(np.float32)
    WoT3 = (inputs["Wo"].T / 3.0).astype(np.float32)      # [768 in, 768 out]
    bq, bk, bv = (np.asarray(inputs[k], np.float32) for k in ("bq", "bk", "bv"))

    grp = []
    for g in range(HG):
        hs = slice(HPG * g * DH, HPG * (g + 1) * DH)
        wq, wk, wv = WqT[:, hs], WkT[:, hs], WvT[:, hs]   # [768, 384]
        Wkv2 = np.empty((D, HPG, 128), np.float32)
        bkv2c = np.empty((128, HPG), np.float32)
        for j in range(HPG):
            js = slice(j * DH, (j + 1) * DH)
            if j % 2 == 0:
                Wkv2[:, j, 0:64], Wkv2[:, j, 64:128] = wk[:, js], wv[:, js]
                bkv2c[0:64, j], bkv2c[64:128, j] = bk[hs][js], bv[hs][js]
            else:
                Wkv2[:, j, 0:64], Wkv2[:, j, 64:128] = wv[:, js], wk[:, js]
                bkv2c[0:64, j], bkv2c[64:128, j] = bv[hs][js], bk[hs][js]
        wo = np.empty((DH, HPG, ECH, 128), np.float32)
        for j in range(HPG):
            for dcc in range(ECH):
                wo[:, j, dcc, :] = WoT3[
                    HPG * g * DH + j * DH : HPG * g * DH + (j + 1) * DH,
                    128 * dcc : 128 * (dcc + 1),
                ]
        xk = np.concatenate(
            [xT[:, topk_idx[HPG * g + j]] for j in range(HPG)], axis=1
        )
        grp.append(
            dict(
                WqTg=_pm(np.ascontiguousarray(wq), bf),
                WkTg=_pm(np.ascontiguousarray(wk), bf),
                WvTg=_pm(np.ascontiguousarray(wv), bf),
                bqg_col=np.ascontiguousarray(
                    bq[hs].reshape(HPG // 2, 128).T, np.float32
                ),
                bkg_col=np.ascontiguousarray(
                    bk[hs].reshape(HPG // 2, 128).T, np.float32
                ),
                bvg=np.ascontiguousarray(bv[hs][None, :]).astype(bf),
                Wkv2=_pm(Wkv2.reshape(D, HPG * 128), bf).reshape(128, ECH, HPG, 128),
                bkv2_col=np.ascontiguousarray(bkv2c),
                Wo3=np.ascontiguousarray(wo.astype(bf)),
                xTk=_pm(xk, bf),
            )
        )

    xTg_pm = _pm(xT[:, :NG], bf)
    p = np.arange(128)[:, None, None]
    kk = np.arange(BCH)[None, :, None]
    sl = np.arange(QB)[None, None, :]
    in_b = []
    for c in range(NCORES):
        qb, g = c // HG, c % HG
        s0 = min(max(4 * qb - 2, 0), S // 128 - BCH)
        M8 = (np.abs(128 * s0 + 128 * kk + p - (QB * qb + sl)) <= LWH).astype(bf)
        in_b.append(
            dict(
                grp[g],
                xTb=_pm(xT[:, 128 * s0 : 128 * s0 + BT], bf),
                xTq=_pm(xT[:, QB * qb : QB * (qb + 1)], bf),
                xTg=xTg_pm,
                M8=np.ascontiguousarray(M8),
            )
        )
    return in_b


def kernel(**inputs):
    x = np.asarray(inputs["x"][0], np.float32)            # [S, D]
    xT = np.ascontiguousarray(x.T)                        # [D, S]
    nc_a, nc_b = _get_progs()

    # ---- phase A: token scores, sharded over 8 cores ----
    WsT_pm = _pm(np.ascontiguousarray(inputs["Ws"].T, np.float32), ml_dtypes.bfloat16)
    bs_row = np.ascontiguousarray(inputs["bs"][None, :]).astype(ml_dtypes.bfloat16)
    TPC = S // NCORES
    in_a = [
        {
            "xTa": _pm(xT[:, TPC * c : TPC * (c + 1)], ml_dtypes.bfloat16),
            "WsT": WsT_pm,
            "bs_row": bs_row,
        }
        for c in range(NCORES)
    ]
    ra = run_bass_kernel_spmd(nc_a, in_a, list(range(NCORES)), trace=TRACE)
    ts = np.concatenate([r["ts"] for r in ra.results], axis=1)  # [H, S]
    LAST_EXEC_NS["phase_a"] = ra.exec_time_ns

    # f32r rounds scores by up to ~2^-11 * |score|; re-evaluate tokens near
    # each head's top-k threshold exactly so the selection matches fp32.
    Ws32 = np.asarray(inputs["Ws"], np.float32)
    bs32 = np.asarray(inputs["bs"], np.float32)
    topk_idx = []
    for h in range(H):
        order = np.argpartition(-ts[h], TOPK)
        thresh = ts[h][order[TOPK - 1]]
        margin = 0.02
        cand = np.nonzero(np.abs(ts[h] - thresh) <= margin)[0]
        tsf = ts[h].copy()
        tsf[cand] = Ws32[h] @ xT[:, cand] + bs32[h]
        topk_idx.append(np.argpartition(-tsf, TOPK)[:TOPK])
    in_b = _inputs_b(inputs, xT, topk_idx)
    res = run_bass_kernel_spmd(nc_b, in_b, list(range(NCORES)), trace=TRACE)
    LAST_EXEC_NS["phase_b"] = res.exec_time_ns

    bo = np.asarray(inputs["bo"], np.float32)
    out = np.empty((S, D), np.float32)
    for qb in range(NQB):
        ypm = res.results[2 * qb]["yT"] + res.results[2 * qb + 1]["yT"]
        yfull = ypm.transpose(1, 0, 2).reshape(D, QB)     # [768, 512]
        out[QB * qb : QB * (qb + 1)] = yfull.T + bo
    return out.reshape(1, S, D)


# revision 57
# speedup vs baseline: 1.0910x; 1.0910x over previous
# Trainium2 Bass kernel for DeepSeek-style sparse attention.
# Self-contained: hardcodes shapes from the problem spec.
#   x [1, 2048, 768]; Wq/Wk/Wv/Wo [768, 768]; biases [768]; Ws [12, 768]; bs [12]
#
# Sharding: 8 cores = 4 query-blocks (512 queries) x 2 head-groups (6 heads).
# Each core computes, for its 512 queries and 6 heads:
#   - band K/Q/V projections over an 8-chunk (1024-token) window that covers
#     the +-256 local band of its query block (host slices x accordingly),
#   - a compact top-k branch from host-gathered x columns (256 per head,
#     indices from a tiny fp32 phase-A token-score kernel + host argpartition),
#   - the 16-token global branch,
#   - a partial out-projection over its 6 heads' dims. The host sums the two
#     head-group partials per query block and adds bo (no device collective).
# Everything post-PSUM runs in bf16; phase A stays fp32 so the top-k
# selection matches the fp32 reference bit-for-bit on near-ties.
import sys
import numpy as np
import ml_dtypes

sys.path.insert(0, "/opt/trn_rl_repo")

import concourse.bass as bass
from concourse import bacc
import concourse.mybir as mybir
from concourse.tile import TileContext
from concourse.bass_utils import run_bass_kernel_spmd
from concourse.masks import make_identity

S = 2048
D = 768
H = 12
DH = 64
NCORES = 8
NQB = 4                    # query blocks
QB = S // NQB              # 512 queries per block
HG = 2                     # head groups
HPG = H // HG              # 6 heads per group
ECH = D // 128             # 6 embedding chunks
BCH = 8                    # band chunks per core (8 x 128 = 1024 tokens)
BT = BCH * 128             # 1024 band tokens
TOPK = 256
NG = 16
LWH = 256                  # local window half-width
SCALE = 1.0 / np.sqrt(DH)
F32 = mybir.dt.float32
BF16 = mybir.dt.bfloat16
FP8 = mybir.dt.float8e4
Exp = mybir.ActivationFunctionType.Exp
Ident = mybir.ActivationFunctionType.Identity
DblRow = mybir.MatmulPerfMode.DoubleRow


def _patch_tile_drain():
    """This walrus build rejects sem-waits on Drain instructions ("Too many
    sync wait commands"). Emit the tail waits as individual SemWait ops on
    the sync engine instead, then a bare drain."""
    if getattr(TileContext, "_drain_patched", False):
        return

    def _drain_and_barrier(self, tick_clock, wait_clock):
        nc = self.nc
        clock = tick_clock.global_clock
        for proc, handle in sorted(self.sems.allocated().items()):
            tick = clock[proc]
            if tick <= 0:
                continue
            mult = 16 if "DMA" in handle.name else 1
            nc.sync.wait_ge(handle, tick * mult)
        nc.sync.drain()
        nc.all_engine_barrier()
        popped = nc._tile_sem_poison_stack.pop()
        assert popped is self._sem_poison
        nc.clear_and_free_semaphores(list(self.sems.allocated().values()))
        nc.all_engine_barrier()

    TileContext._drain_and_barrier = _drain_and_barrier
    TileContext._drain_patched = True


def _build_phase_a():
    """ts[h, 256] = (Ws @ x^T + bs) for this core's 256-token slice.

    Runs in bf16; the host re-evaluates tokens near the top-k threshold in
    exact fp32, so rounding cannot flip the selection."""
    _patch_tile_drain()
    nc = bacc.Bacc()
    TPC = S // NCORES  # 256 tokens per core
    xTa = nc.declare_dram_parameter("xTa", [128, ECH, TPC], BF16, isOutput=False)
    WsT = nc.declare_dram_parameter("WsT", [128, ECH, H], BF16, isOutput=False)
    bs_row = nc.declare_dram_parameter("bs_row", [1, H], BF16, isOutput=False)
    ts = nc.declare_dram_parameter("ts", [H, TPC], F32, isOutput=True)

    with TileContext(nc) as tc, nc.allow_low_precision(reason="host refines boundary"):
        with (
            tc.tile_pool(name="sb", bufs=1) as sb,
            tc.tile_pool(name="ps", bufs=1, space="PSUM") as ps,
        ):
            xTa_sb = sb.tile([128, ECH, TPC], BF16)
            WsT_sb = sb.tile([128, ECH, H], BF16)
            bs_sb = sb.tile([1, H], BF16)
            ones = sb.tile([1, TPC], BF16)
            nc.vector.memset(ones, 1.0)
            nc.sync.dma_start(out=bs_sb, in_=bs_row[:, :])
            nc.sync.dma_start(out=WsT_sb, in_=WsT[:, :, :])
            nc.sync.dma_start(out=xTa_sb, in_=xTa[:, :, :])
            acc = ps.tile([H, TPC], F32)
            for ec in range(ECH):
                nc.tensor.matmul(
                    acc, WsT_sb[:, ec, :], xTa_sb[:, ec, :],
                    start=(ec == 0), stop=False,
                )
            nc.tensor.matmul(acc, bs_sb, ones, start=False, stop=True)
            ts_sb = sb.tile([H, TPC], F32)
            nc.vector.tensor_copy(ts_sb, acc)
            nc.sync.dma_start(out=ts[:, :], in_=ts_sb)
    nc.finalize()
    return nc


def _build_phase_b(debug=False):
    """Per-core sparse attention for 512 queries x 6 heads (see header)."""
    _patch_tile_drain()
    nc = bacc.Bacc()
    # host-prepared, partition-major, bf16
    xTb = nc.declare_dram_parameter("xTb", [128, ECH, BT], BF16, isOutput=False)
    xTq = nc.declare_dram_parameter("xTq", [128, ECH, QB], BF16, isOutput=False)
    xTk = nc.declare_dram_parameter("xTk", [128, ECH, HPG * TOPK], BF16, isOutput=False)
    xTg = nc.declare_dram_parameter("xTg", [128, ECH, NG], BF16, isOutput=False)
    # per-type weight slices for this head group (384 = 6 heads x 64 dims)
    WqTg = nc.declare_dram_parameter("WqTg", [128, ECH, 384], BF16, isOutput=False)
    WkTg = nc.declare_dram_parameter("WkTg", [128, ECH, 384], BF16, isOutput=False)
    WvTg = nc.declare_dram_parameter("WvTg", [128, ECH, 384], BF16, isOutput=False)
    # per-partition bias columns, added during the PSUM->SBUF copies
    bqg_col = nc.declare_dram_parameter("bqg_col", [128, HPG // 2], F32, isOutput=False)
    bkg_col = nc.declare_dram_parameter("bkg_col", [128, HPG // 2], F32, isOutput=False)
    bkv2_col = nc.declare_dram_parameter("bkv2_col", [128, HPG], F32, isOutput=False)
    bvg = nc.declare_dram_parameter("bvg", [1, 384], BF16, isOutput=False)
    # stacked [Wk_h | Wv_h] (even local head) / [Wv_h | Wk_h] (odd) for the
    # compact top-k / global projections
    Wkv2 = nc.declare_dram_parameter("Wkv2", [128, ECH, HPG, 128], BF16, isOutput=False)
    # Wo^T/3 slices: wo[dh, j, dc, :] over this group's heads
    Wo3 = nc.declare_dram_parameter("Wo3", [DH, HPG, ECH, 128], BF16, isOutput=False)
    M8 = nc.declare_dram_parameter("M8", [128, BCH, QB], BF16, isOutput=False)
    yT = nc.declare_dram_parameter("yT", [128, ECH, QB], F32, isOutput=True)

    with TileContext(nc) as tc, nc.allow_low_precision(reason="bf16 validated vs reference"):
        with tc.tile_pool(name="perm", bufs=1) as perm:
            kT_sb = perm.tile([128, HPG // 2, BT], BF16)       # 2 heads / 128 parts
            qT_sb = perm.tile([128, HPG // 2, QB], BF16)
            ktk_sb = perm.tile([128, HPG // 2, TOPK], BF16)
            vtkT_sb = perm.tile([128, HPG // 2, TOPK], BF16)   # staging (pre-transpose)
            vtk_sb = perm.tile([128, HPG, TOPK // 128, DH + 1], BF16)
            kg_sb = perm.tile([128, HPG // 2, NG], BF16)
            vgT_sb = perm.tile([128, HPG // 2, NG], BF16)      # staging
            vg_sb = perm.tile([NG, HPG, DH + 1], BF16)
            v_sb = perm.tile([128, HPG, BCH, DH + 1], BF16)
            M8_sb = perm.tile([128, BCH, QB], BF16)
            attnT = perm.tile([DH, HPG, QB], BF16)
            yT_sb = perm.tile([128, ECH, QB], F32)
            wo_sb = perm.tile([DH, HPG, ECH, 128], BF16)
            bqg_sb = perm.tile([128, HPG // 2], F32)
            bkg_sb = perm.tile([128, HPG // 2], F32)
            bvg_sb = perm.tile([1, 384], BF16)
            bkv2_sb = perm.tile([128, HPG], F32)
            ones = perm.tile([1, QB], BF16)
            id128 = perm.tile([128, 128], BF16)
            nc.vector.memset(ones, 1.0)
            make_identity(nc, id128)
            nc.vector.memset(v_sb[:, :, :, DH : DH + 1], 1.0)
            nc.vector.memset(vtk_sb[:, :, :, DH : DH + 1], 1.0)
            nc.vector.memset(vg_sb[:, :, DH : DH + 1], 1.0)
            nc.sync.dma_start(out=bqg_sb, in_=bqg_col[:, :])
            nc.sync.dma_start(out=bkg_sb, in_=bkg_col[:, :])
            nc.sync.dma_start(out=bvg_sb, in_=bvg[:, :])
            nc.sync.dma_start(out=bkv2_sb, in_=bkv2_col[:, :])

            with (
                tc.tile_pool(name="xin", bufs=1) as xin,
                tc.tile_pool(name="pj_ps", bufs=4, space="PSUM") as pj_ps,
                tc.tile_pool(name="pg_ps", bufs=2, space="PSUM") as pg_ps,
                tc.tile_pool(name="pt_ps", bufs=2, space="PSUM") as pt_ps,
            ):
                xTb_sb = xin.tile([128, ECH, BT], BF16)
                xTq_sb = xin.tile([128, ECH, QB], BF16)
                xTk_sb = xin.tile([128, ECH, HPG * TOPK], BF16)
                xTg_sb = xin.tile([128, ECH, NG], BF16)
                WqTg_sb = xin.tile([128, ECH, 384], BF16)
                WkTg_sb = xin.tile([128, ECH, 384], BF16)
                WvTg_sb = xin.tile([128, ECH, 384], BF16)
                Wkv2_sb = xin.tile([128, ECH, HPG, 128], BF16)
                # Whole-tile DMAs (one descriptor per partition — per-chunk
                # splits multiply descriptor count and serialize the queues).
                # Q first so the PE starts while K/V/band stream in.
                nc.sync.dma_start(out=WqTg_sb, in_=WqTg[:, :, :])
                nc.sync.dma_start(out=xTq_sb, in_=xTq[:, :, :])
                nc.sync.dma_start(out=WkTg_sb, in_=WkTg[:, :, :])
                nc.sync.dma_start(out=xTb_sb, in_=xTb[:, :, :])
                nc.sync.dma_start(out=WvTg_sb, in_=WvTg[:, :, :])
                nc.sync.dma_start(out=Wkv2_sb, in_=Wkv2[:, :, :, :])
                nc.sync.dma_start(out=xTk_sb, in_=xTk[:, :, :])
                nc.sync.dma_start(out=xTg_sb, in_=xTg[:, :, :])
                # late-phase inputs last so they don't delay the projections
                nc.sync.dma_start(out=M8_sb, in_=M8[:, :, :])
                nc.sync.dma_start(out=wo_sb, in_=Wo3[:, :, :, :])

                # ---- Q^T [dims, 512], then band K^T [dims, 1024] ----
                # (biases added during the PSUM->SBUF copy on the ACT engine)
                for dc in range(HPG // 2):
                    ds = slice(128 * dc, 128 * (dc + 1))
                    qp = pj_ps.tile([128, 512], F32, tag="pj")
                    for ec in range(ECH):
                        nc.tensor.matmul(
                            qp, WqTg_sb[:, ec, ds], xTq_sb[:, ec, :],
                            start=(ec == 0), stop=(ec == ECH - 1),
                        )
                    nc.scalar.activation(
                        qT_sb[:, dc, :], qp, Ident, bias=bqg_sb[:, dc : dc + 1]
                    )
                for dc in range(HPG // 2):
                    ds = slice(128 * dc, 128 * (dc + 1))
                    for half in range(2):
                        t0 = 512 * half
                        kp = pj_ps.tile([128, 512], F32, tag="pj")
                        for ec in range(ECH):
                            nc.tensor.matmul(
                                kp, WkTg_sb[:, ec, ds],
                                xTb_sb[:, ec, t0 : t0 + 512],
                                start=(ec == 0), stop=(ec == ECH - 1),
                            )
                        nc.scalar.activation(
                            kT_sb[:, dc, t0 : t0 + 512], kp, Ident,
                            bias=bkg_sb[:, dc : dc + 1],
                        )

                # ---- band V [1024 tokens, 6 heads x 64] ----
                for tcn in range(BCH):
                    t0 = 128 * tcn
                    vp = pj_ps.tile([128, HPG * DH], F32, tag="pj")
                    for ec in range(ECH):
                        nc.tensor.matmul(
                            vp, xTb_sb[:, ec, t0 : t0 + 128],
                            WvTg_sb[:, ec, :],
                            start=(ec == 0), stop=False,
                        )
                    nc.tensor.matmul(
                        vp, ones[:, :128], bvg_sb,
                        start=False, stop=True,
                    )
                    nc.any.tensor_copy(
                        v_sb[:, :, tcn, 0:DH],
                        vp.rearrange("p (h d) -> p h d", d=DH),
                    )

                # ---- compact top-k + global K/V per head (stacked K|V) ----
                for j in range(HPG):
                    hp = (j % 2) * 64
                    dc = j // 2
                    sp = pj_ps.tile([128, TOPK], F32, tag="pj")
                    sg = pg_ps.tile([128, NG], F32, tag="pg")
                    for ec in range(ECH):
                        nc.tensor.matmul(
                            sp, Wkv2_sb[:, ec, j, :],
                            xTk_sb[:, ec, TOPK * j : TOPK * (j + 1)],
                            start=(ec == 0), stop=(ec == ECH - 1),
                        )
                        nc.tensor.matmul(
                            sg, Wkv2_sb[:, ec, j, :], xTg_sb[:, ec, :],
                            start=(ec == 0), stop=(ec == ECH - 1),
                        )
                    # even j: K at rows 0:64, V^T at 64:128; odd j: swapped
                    bj = bkv2_sb[:, j : j + 1]
                    nc.scalar.activation(
                        ktk_sb[hp : hp + 64, dc, :], sp[hp : hp + 64, :],
                        Ident, bias=bj[hp : hp + 64, :],
                    )
                    nc.scalar.activation(
                        kg_sb[hp : hp + 64, dc, :], sg[hp : hp + 64, :],
                        Ident, bias=bj[hp : hp + 64, :],
                    )
                    vq = 64 - hp
                    nc.scalar.activation(
                        vtkT_sb[vq : vq + 64, dc, :], sp[vq : vq + 64, :],
                        Ident, bias=bj[vq : vq + 64, :],
                    )
                    nc.scalar.activation(
                        vgT_sb[vq : vq + 64, dc, :], sg[vq : vq + 64, :],
                        Ident, bias=bj[vq : vq + 64, :],
                    )

                # ---- transpose V^T staging into [token, dh] layout ----
                for j in range(HPG):
                    vq = 64 - (j % 2) * 64
                    dc = j // 2
                    idsl = id128[vq : vq + 64, vq : vq + 64]
                    for c in range(TOPK // 128):
                        tp = pt_ps.tile([128, 64], BF16, tag="pt")
                        nc.tensor.transpose(
                            tp, vtkT_sb[vq : vq + 64, dc, 128 * c : 128 * (c + 1)],
                            idsl,
                        )
                        nc.any.tensor_copy(vtk_sb[:, j, c, 0:DH], tp)
                    tg = pt_ps.tile([128, 64], BF16, tag="pt")
                    nc.tensor.transpose(tg[0:NG, :], vgT_sb[vq : vq + 64, dc, :NG], idsl)
                    nc.any.tensor_copy(vg_sb[:, j, 0:DH], tg[0:NG, :])

            # ---- per-head attention ----
            with (
                tc.tile_pool(name="attn", bufs=2) as attn,
                tc.tile_pool(name="nrm", bufs=2) as nrm,
                tc.tile_pool(name="st_ps", bufs=2, space="PSUM") as st_ps,
                tc.tile_pool(name="av_ps", bufs=1, space="PSUM") as av_ps,
            ):
                for j in range(HPG):
                    hp = (j % 2) * 64
                    dc = j // 2
                    kTh = kT_sb[hp : hp + 64, dc, :]
                    qTh = qT_sb[hp : hp + 64, dc, :]
                    # scores + exp, global first so its AV/norm chain starts
                    # earliest; band mask folded per round so AV streams.
                    stg = st_ps.tile([128, 2, QB], F32, tag="st")
                    nc.tensor.matmul(
                        stg[0:NG, 0, :], kg_sb[hp : hp + 64, dc, :], qTh,
                        start=True, stop=True,
                    )
                    ETg = attn.tile([NG, QB], BF16, tag="ETg")
                    nc.scalar.activation(ETg, stg[0:NG, 0, :], Exp, scale=SCALE)
                    stk = st_ps.tile([128, 2, QB], F32, tag="st")
                    for c in range(2):
                        nc.tensor.matmul(
                            stk[:, c, :], ktk_sb[hp : hp + 64, dc, 128 * c : 128 * (c + 1)],
                            qTh, start=True, stop=True,
                        )
                    ETk = attn.tile([128, 2, QB], BF16, tag="ETk")
                    nc.scalar.activation(ETk, stk, Exp, scale=SCALE)
                    ET = attn.tile([128, BCH, QB], BF16, tag="ET")
                    for rnd in range(BCH // 2):
                        stp = st_ps.tile([128, 2, QB], F32, tag="st")
                        for i in range(2):
                            cc = 2 * rnd + i
                            nc.tensor.matmul(
                                stp[:, i, :], kTh[:, 128 * cc : 128 * (cc + 1)],
                                qTh, start=True, stop=True,
                            )
                        nc.scalar.activation(
                            ET[:, 2 * rnd : 2 * rnd + 2, :], stp, Exp, scale=SCALE
                        )
                        nc.vector.tensor_mul(
                            ET[:, 2 * rnd : 2 * rnd + 2, :],
                            ET[:, 2 * rnd : 2 * rnd + 2, :],
                            M8_sb[:, 2 * rnd : 2 * rnd + 2, :],
                        )
                    # per-branch AV -> copy out -> denom row to partition 0
                    # (partition_broadcast only reads partition 0 on HW) ->
                    # broadcast -> reciprocal. Chains run off the PE and
                    # overlap the next branch/head.
                    av = av_ps.tile([DH + 1, 3, QB], F32, tag="av")
                    avs = nrm.tile([DH + 1, 3, QB], F32, tag="avs")
                    dn0 = nrm.tile([1, 3, QB], F32, tag="dn0")
                    dbs = nrm.tile([DH, 3, QB], F32, tag="dbs")
                    rbs = nrm.tile([DH, 3, QB], F32, tag="rbs")

                    def branch_norm(b):
                        nc.scalar.copy(avs[:, b, :], av[:, b, :])
                        nc.sync.dma_start(
                            out=dn0[:, b, :], in_=avs[DH : DH + 1, b, :]
                        )
                        nc.gpsimd.partition_broadcast(dbs[:, b, :], dn0[:, b, :])
                        nc.vector.reciprocal_approx_fast(rbs[:, b, :], dbs[:, b, :])

                    nc.tensor.matmul(
                        av[:, 2, :], vg_sb[:, j, :], ETg, start=True, stop=True
                    )
                    branch_norm(2)
                    for c in range(TOPK // 128):
                        nc.tensor.matmul(
                            av[:, 1, :], vtk_sb[:, j, c, :], ETk[:, c, :],
                            start=(c == 0), stop=(c == TOPK // 128 - 1),
                        )
                    branch_norm(1)
                    for tcn in range(BCH):
                        nc.tensor.matmul(
                            av[:, 0, :], v_sb[:, j, tcn, :], ET[:, tcn, :],
                            start=(tcn == 0), stop=(tcn == BCH - 1),
                        )
                    branch_norm(0)
                    ta = nrm.tile([DH, QB], F32, tag="ta")
                    tb = nrm.tile([DH, QB], F32, tag="tb")
                    nc.vector.tensor_mul(ta, avs[0:DH, 2, :], rbs[:, 2, :])
                    nc.vector.tensor_mul(tb, avs[0:DH, 1, :], rbs[:, 1, :])
                    nc.vector.tensor_add(ta, ta, tb)
                    nc.vector.tensor_mul(tb, avs[0:DH, 0, :], rbs[:, 0, :])
                    nc.vector.tensor_add(attnT[:, j, :], ta, tb)

            # ---- partial out-projection (Wo/3 folded on host) ----
            with tc.tile_pool(name="yt_ps", bufs=2, space="PSUM") as yt_ps:
                for ddc in range(ECH):
                    yp = yt_ps.tile([128, QB], F32, tag="yt")
                    for j in range(HPG):
                        nc.tensor.matmul(
                            yp, wo_sb[:, j, ddc, :], attnT[:, j, :],
                            start=(j == 0), stop=(j == HPG - 1),
                        )
                    nc.any.tensor_copy(yT_sb[:, ddc, :], yp)
                    nc.sync.dma_start(out=yT[:, ddc, :], in_=yT_sb[:, ddc, :])
    nc.finalize()
    return nc


_PROGS = {}
TRACE = False
LAST_EXEC_NS = {}


def _get_progs():
    if "a" not in _PROGS:
        _PROGS["a"] = _build_phase_a()
        _PROGS["b"] = _build_phase_b()
    return _PROGS["a"], _PROGS["b"]


def _pm(arr, dtype):
    """[768, T] -> partition-major [128, 6, T] contiguous."""
    d, t = arr.shape
    assert d == D
    return np.ascontiguousarray(
        arr.reshape(ECH, 128, t).transpose(1, 0, 2).astype(dtype)
    )


def _inputs_b(inputs, xT, topk_idx):
    """Build the 8 per-core phase-B input maps."""
    bf = ml_dtypes.bfloat16
    WqT = inputs["Wq"].T.astype(np.float32)
    WkT = inputs["Wk"].T.astype(np.float32)
    WvT = inputs["Wv"].T.astype(np.float32)
    WoT3 = (inputs["Wo"].T / 3.0).astype(np.float32)      # [768 in, 768 out]
    bq, bk, bv = (np.asarray(inputs[k], np.float32) for k in ("bq", "bk", "bv"))

    grp = []
    for g in range(HG):
        hs = slice(HPG * g * DH, HPG * (g + 1) * DH)
        wq, wk, wv = WqT[:, hs], WkT[:, hs], WvT[:, hs]   # [768, 384]
        Wkv2 = np.empty((D, HPG, 128), np.float32)
        bkv2c = np.empty((128, HPG), np.float32)
        for j in range(HPG):
            js = slice(j * DH, (j + 1) * DH)
            if j % 2 == 0:
                Wkv2[:, j, 0:64], Wkv2[:, j, 64:128] = wk[:, js], wv[:, js]
                bkv2c[0:64, j], bkv2c[64:128, j] = bk[hs][js], bv[hs][js]
            else:
                Wkv2[:, j, 0:64], Wkv2[:, j, 64:128] = wv[:, js], wk[:, js]
                bkv2c[0:64, j], bkv2c[64:128, j] = bv[hs][js], bk[hs][js]
        wo = np.empty((DH, HPG, ECH, 128), np.float32)
        for j in range(HPG):
            for dcc in range(ECH):
                wo[:, j, dcc, :] = WoT3[
                    HPG * g * DH + j * DH : HPG * g * DH + (j + 1) * DH,
                    128 * dcc : 128 * (dcc + 1),
                ]
        xk = np.concatenate(
            [xT[:, topk_idx[HPG * g + j]] for j in range(HPG)], axis=1
        )
        grp.append(
            dict(
                WqTg=_pm(np.ascontiguousarray(wq), bf),
                WkTg=_pm(np.ascontiguousarray(wk), bf),
                WvTg=_pm(np.ascontiguousarray(wv), bf),
                bqg_col=np.ascontiguousarray(
                    bq[hs].reshape(HPG // 2, 128).T, np.float32
                ),
                bkg_col=np.ascontiguousarray(
                    bk[hs].reshape(HPG // 2, 128).T, np.float32
                ),
                bvg=np.ascontiguousarray(bv[hs][None, :]).astype(bf),
                Wkv2=_pm(Wkv2.reshape(D, HPG * 128), bf).reshape(128, ECH, HPG, 128),
                bkv2_col=np.ascontiguousarray(bkv2c),
                Wo3=np.ascontiguousarray(wo.astype(bf)),
                xTk=_pm(xk, bf),
            )
        )

    xTg_pm = _pm(xT[:, :NG], bf)
    p = np.arange(128)[:, None, None]
    kk = np.arange(BCH)[None, :, None]
    sl = np.arange(QB)[None, None, :]
    in_b = []
    for c in range(NCORES):
        qb, g = c // HG, c % HG
        s0 = min(max(4 * qb - 2, 0), S // 128 - BCH)
        M8 = (np.abs(128 * s0 + 128 * kk + p - (QB * qb + sl)) <= LWH).astype(bf)
        in_b.append(
            dict(
                grp[g],
                xTb=_pm(xT[:, 128 * s0 : 128 * s0 + BT], bf),
                xTq=_pm(xT[:, QB * qb : QB * (qb + 1)], bf),
                xTg=xTg_pm,
                M8=np.ascontiguousarray(M8),
            )
        )
    return in_b


def kernel(**inputs):
    x = np.asarray(inputs["x"][0], np.float32)            # [S, D]
    xT = np.ascontiguousarray(x.T)                        # [D, S]
    nc_a, nc_b = _get_progs()

    # ---- phase A: token scores, sharded over 8 cores ----
    WsT_pm = _pm(np.ascontiguousarray(inputs["Ws"].T, np.float32), ml_dtypes.bfloat16)
    bs_row = np.ascontiguousarray(inputs["bs"][None, :]).astype(ml_dtypes.bfloat16)
    TPC = S // NCORES
    in_a = [
        {
            "xTa": _pm(xT[:, TPC * c : TPC * (c + 1)], ml_dtypes.bfloat16),
            "WsT": WsT_pm,
            "bs_row": bs_row,
        }
        for c in range(NCORES)
    ]
    ra = run_bass_kernel_spmd(nc_a, in_a, list(range(NCORES)), trace=TRACE)
    ts = np.concatenate([r["ts"] for r in ra.results], axis=1)  # [H, S]
    LAST_EXEC_NS["phase_a"] = ra.exec_time_ns

    # f32r rounds scores by up to ~2^-11 * |score|; re-evaluate tokens near
    # each head's top-k threshold exactly so the selection matches fp32.
    Ws32 = np.asarray(inputs["Ws"], np.float32)
    bs32 = np.asarray(inputs["bs"], np.float32)
    topk_idx = []
    for h in range(H):
        order = np.argpartition(-ts[h], TOPK)
        thresh = ts[h][order[TOPK - 1]]
        margin = 0.02
        cand = np.nonzero(np.abs(ts[h] - thresh) <= margin)[0]
        tsf = ts[h].copy()
        tsf[cand] = Ws32[h] @ xT[:, cand] + bs32[h]
        topk_idx.append(np.argpartition(-tsf, TOPK)[:TOPK])
    in_b = _inputs_b(inputs, xT, topk_idx)
    res = run_bass_kernel_spmd(nc_b, in_b, list(range(NCORES)), trace=TRACE)
    LAST_EXEC_NS["phase_b"] = res.exec_time_ns

    bo = np.asarray(inputs["bo"], np.float32)
    out = np.empty((S, D), np.float32)
    for qb in range(NQB):
        ypm = res.results[2 * qb]["yT"] + res.results[2 * qb + 1]["yT"]
        yfull = ypm.transpose(1, 0, 2).reshape(D, QB)     # [768, 512]
        out[QB * qb : QB * (qb + 1)] = yfull.T + bo
    return out.reshape(1, S, D)
